# revision 1
# baseline (speedup 1.0000x reference)
"""Trainium2 Bass kernel: conv3x3 + channel attention (4 heads) + 1x1 proj.

Data-parallel over batch: 8 batch elements -> 8 NeuronCores, no collectives.

Algorithm per core (one batch element, C=128, H=W=128, N=H*W=16384):
  xl = conv3x3(x)                                   # 9 shifted matmuls, bf16
  G  = xl @ xl^T          [128,128]                 # xlT via DMA-xbar transpose
  # channel attention factors through G:
  #   gram_qq|qk = Wq G Wq^T | Wq G Wk^T ; gram_kk = Wk G Wk^T
  #   attn = softmax(gram_qk[c,d] / (nq[c] nk[d]) + head mask)
  #   y = Wp @ blockdiag(attn) @ Wv @ xl = E @ xl   # one combined 128x128 matrix
Bulk matmuls bf16; norms/softmax fp32. ACT uses only {Copy, Ln, Exp} so a
single activation-table set (natural_log_exp_and_others) is loaded once.
"""

import numpy as np
import ml_dtypes

import concourse.bass as bass
import concourse.mybir as mybir
import concourse.tile as tile
from concourse import bacc
from concourse.masks import make_identity

# Make Exp/Ln resolve to the combined "natural_log_exp_and_others" activation
# table set so the whole kernel needs exactly one ACT_TABLE_LOAD (Copy is in
# every set). The default first-match choice picks disjoint sets for Exp and
# Ln, costing two ~1.3us mid-kernel table reloads on the critical path.
_ORIG_GAT = bacc.get_activation_tables


def _gat_one_set(arch):
    tables = _ORIG_GAT(arch)
    for name, fns in tables.items():
        if name != "natural_log_exp_and_others":
            fns.discard(mybir.ActivationFunctionType.Exp)
            fns.discard(mybir.ActivationFunctionType.Ln)
    return tables


bacc.get_activation_tables = _gat_one_set

P = 128
H = W = 128
HP, WP = H + 2, W + 2          # zero-padded input
NPOS = H * W                   # 16384
CHUNK = 512                    # spatial chunk (4 rows)
NCHUNK = NPOS // CHUNK         # 32
NSTRIP = 4                     # input strips (with halo)
STRIP_ROWS = 34                # 32 rows + 2 halo
NH = 4                         # heads
CH = 32                        # channels per head
BF = mybir.dt.bfloat16
F32 = mybir.dt.float32
AX = mybir.AxisListType
AF = mybir.ActivationFunctionType
OP = mybir.AluOpType
MASK_NEG = -1.0e12
N_WARMUP = 16                  # PE warm-up matmuls hidden under the input DMA
G_LAG = 2                      # chunks of lag before G matmuls consume xlT


def _build():
    nc = bacc.Bacc()
    xp = nc.declare_dram_parameter("xp", [P, HP, WP], BF, isOutput=False)
    wl = nc.declare_dram_parameter("wl", [P, 9, P], BF, isOutput=False)
    wqk = nc.declare_dram_parameter("wqk", [P, 2 * P], BF, isOutput=False)
    wv = nc.declare_dram_parameter("wv", [P, P], BF, isOutput=False)
    wp = nc.declare_dram_parameter("wp", [P, P], BF, isOutput=False)
    out = nc.declare_dram_parameter("out", [P, NPOS], BF, isOutput=True)

    with tile.TileContext(nc) as tc:
        with (
            tc.tile_pool(name="consts", bufs=1) as consts,
            tc.tile_pool(name="xstrip", bufs=NSTRIP) as xstrip_pool,
            tc.tile_pool(name="xl", bufs=NCHUNK) as xl_pool,
            tc.tile_pool(name="xlt", bufs=G_LAG + 2) as xlt_pool,
            tc.tile_pool(name="small", bufs=1) as small,
            tc.tile_pool(name="ysb", bufs=4) as y_pool,
            tc.tile_pool(name="ps512", bufs=3, space="PSUM") as ps512,
            tc.tile_pool(name="psT", bufs=2, space="PSUM") as psT,
            tc.tile_pool(name="psG", bufs=1, space="PSUM") as psG,
            tc.tile_pool(name="psB2", bufs=2, space="PSUM") as psB2,
        ):
            # ---- conv weights + input strips first so their DMAs lead the queue ----
            # First strip is split in half (18 rows each) so the first conv
            # chunks can start ~2us earlier; chunks 0-3 need xp rows 0-17 only.
            strip_defs = [(0, 18), (16, 18)] + [
                (32 * s, STRIP_ROWS) for s in range(1, NSTRIP)
            ]
            st0 = xstrip_pool.tile([P, 18, WP], BF, tag="xstrip0")
            nc.sync.dma_start(out=st0[:], in_=xp[:, 0:18, :])
            wl_sb = consts.tile([P, 9, P], BF, tag="wl")
            nc.sync.dma_start(out=wl_sb[:], in_=wl[:])
            xstrips = [st0]
            for r0, nr in strip_defs[1:]:
                st = xstrip_pool.tile([P, STRIP_ROWS, WP], BF, tag="xstrip")
                nc.sync.dma_start(out=st[:, 0:nr, :], in_=xp[:, r0: r0 + nr, :])
                xstrips.append(st)

            def conv_src(c):
                # returns (strip tile, local row base) for output chunk c
                if c < 4:
                    return xstrips[0], 4 * c
                if c < 8:
                    return xstrips[1], 4 * c - 16
                s = c // 8
                return xstrips[s + 1], 4 * (c % 8)
            wqk_sb = consts.tile([P, 2 * P], BF, tag="wqk")
            nc.sync.dma_start(out=wqk_sb[:], in_=wqk[:])
            wv_sb = consts.tile([P, P], BF, tag="wv")
            nc.sync.dma_start(out=wv_sb[:], in_=wv[:])
            wp_sb = consts.tile([P, P], BF, tag="wp")
            nc.sync.dma_start(out=wp_sb[:], in_=wp[:])

            # ---- PE warm-up: junk matmuls with no DMA dependency ----
            junk = consts.tile([P, CHUNK], BF, tag="junk")
            nc.vector.memset(junk[:], 0.125)
            ps_warm = ps512.tile([P, CHUNK], F32, tag="ps512")
            for i in range(N_WARMUP):
                nc.tensor.matmul(ps_warm[:], junk[:, 0:P], junk[:],
                                 start=True, stop=True)

            def keep_warm(n=1):
                pw = ps512.tile([P, CHUNK], F32, tag="ps512")
                for _ in range(n):
                    nc.tensor.matmul(pw[:], junk[:, 0:P], junk[:],
                                     start=True, stop=True)

            # ---- single ACT table load (set: natural_log_exp_and_others) ----
            tl = small.tile([P, 1], F32, tag="tl")
            nc.vector.memset(tl[:], 1.0)
            nc.scalar.activation(tl[:], tl[:], AF.Exp)

            id_bf = consts.tile([P, P], BF, tag="id_bf")
            make_identity(nc, id_bf[:])
            id_f32 = consts.tile([P, P], F32, tag="id_f32")
            make_identity(nc, id_f32[:])
            mask_sb = consts.tile([P, P], F32, tag="mask")
            nc.vector.memset(mask_sb[:], MASK_NEG)
            for h in range(NH):
                nc.vector.memset(mask_sb[h * CH:(h + 1) * CH, h * CH:(h + 1) * CH], 0.0)
            # mask3: identity in slices 0 and 2 (diag extract for qq and kk)
            mask3 = consts.tile([P, 3, P], F32, tag="mask3")
            nc.vector.memset(mask3[:], 0.0)
            nc.gpsimd.tensor_copy(out=mask3[:, 0, :], in_=id_f32[:])
            nc.gpsimd.tensor_copy(out=mask3[:, 2, :], in_=id_f32[:])

            # ---- main loop: conv; xlT via DMA transpose; lagged G accumulation ----
            G_ps = psG.tile([P, P], F32, tag="G")
            xl_tiles = []
            xlt_tiles = []

            def g_mms(ci):
                xlt4 = xlt_tiles[ci]
                for sub in range(4):
                    idx = ci * 4 + sub
                    nc.tensor.matmul(G_ps[:], xlt4[:, sub, :], xlt4[:, sub, :],
                                     start=(idx == 0), stop=(idx == 4 * NCHUNK - 1))

            for c in range(NCHUNK):
                strip, lb = conv_src(c)
                ps_conv = ps512.tile([P, CHUNK], F32, tag="ps512")
                for t in range(9):
                    ky, kx = divmod(t, 3)
                    rhs = strip[:, lb + ky: lb + ky + 4, kx: kx + W]
                    nc.tensor.matmul(ps_conv[:], wl_sb[:, t, :], rhs,
                                     start=(t == 0), stop=(t == 8))
                xl_c = xl_pool.tile([P, CHUNK], BF, tag="xl")
                if c % 2 == 0:
                    nc.scalar.copy(out=xl_c[:], in_=ps_conv[:])
                else:
                    nc.vector.tensor_copy(out=xl_c[:], in_=ps_conv[:])
                xl_tiles.append(xl_c)
                ps_t4 = psT.tile([P, 4, P], F32, tag="psT")
                for sub in range(4):
                    nc.tensor.matmul(ps_t4[:, sub, :], xl_c[:, sub * P:(sub + 1) * P],
                                     id_bf[:], start=True, stop=True)
                xlt4 = xlt_pool.tile([P, 4, P], BF, tag="xlt")
                if c % 2 == 0:
                    nc.vector.tensor_copy(out=xlt4[:], in_=ps_t4[:])
                else:
                    nc.scalar.copy(out=xlt4[:], in_=ps_t4[:])
                xlt_tiles.append(xlt4)
                if c >= G_LAG:
                    g_mms(c - G_LAG)
            for ci in range(NCHUNK - G_LAG, NCHUNK):
                g_mms(ci)

            # ---- attention (128x128; grams bf16, softmax fp32) ----
            G_sb = small.tile([P, P], BF, tag="G_sb")
            nc.vector.tensor_copy(out=G_sb[:], in_=G_ps[:])
            keep_warm()
            # A^T = [A1T | A2T] : out[i',o] = sum_i G[i,i'] WqkT[i,o]
            A_ps = psB2.tile([P, 2 * P], F32, tag="psB2")
            nc.tensor.matmul(A_ps[:], G_sb[:], wqk_sb[:], start=True, stop=True)
            A_sb = small.tile([P, 2 * P], BF, tag="A_sb")
            nc.vector.tensor_copy(out=A_sb[:], in_=A_ps[:])
            keep_warm()
            # grams: [qq | qk] and kk -> layout [c, 3, 128]
            gram_ps = psB2.tile([P, 3 * P], F32, tag="psB2")
            nc.tensor.matmul(gram_ps[:, 0:2 * P], A_sb[:, 0:P], wqk_sb[:],
                             start=True, stop=True)
            nc.tensor.matmul(gram_ps[:, 2 * P:3 * P], A_sb[:, P:2 * P], wqk_sb[:, P:2 * P],
                             start=True, stop=True)
            gram3 = gram_ps[:].rearrange("p (s c) -> p s c", s=3)
            keep_warm()

            # r = 1/sqrt(diag): both diags in one mult+reduce, then exp(-0.5*ln(d))
            # (diag + qk slice are read straight from PSUM)
            dummy3 = small.tile([P, 3, P], F32, tag="dummy3")
            d3 = small.tile([P, 3], F32, tag="d3")
            nc.vector.tensor_tensor(dummy3[:], gram3, mask3[:], OP.mult)
            nc.vector.reduce_sum(d3[:], dummy3[:], axis=AX.X)
            nc.scalar.activation(d3[:], d3[:], AF.Ln)
            r3 = small.tile([P, 3], F32, tag="r3")
            nc.scalar.activation(r3[:], d3[:], AF.Exp, scale=-0.5)
            rq = r3[:, 0:1]
            rk = r3[:, 2:3]
            keep_warm(2)

            # logits = qk * rq[c] * rk[d] + mask  (row scale, transpose, row scale, transpose)
            s1 = small.tile([P, P], F32, tag="s1")
            nc.vector.tensor_scalar_mul(s1[:], gram3[:, 1, :], rq)
            t1_ps = psB2.tile([P, P], F32, tag="psB2")
            nc.tensor.transpose(t1_ps[:], s1[:], id_f32[:])
            s2 = small.tile([P, P], F32, tag="s2")
            nc.vector.tensor_scalar_mul(s2[:], t1_ps[:], rk)
            keep_warm()
            t2_ps = psB2.tile([P, P], F32, tag="psB2")
            nc.tensor.transpose(t2_ps[:], s2[:], id_f32[:])
            lgt = small.tile([P, P], F32, tag="lgt")
            nc.vector.tensor_tensor(lgt[:], t2_ps[:], mask_sb[:], OP.add)
            keep_warm()

            # softmax over free dim (logits <= 1, no max subtraction needed)
            attn = small.tile([P, P], F32, tag="attn")
            rsum = small.tile([P, 1], F32, tag="rsum")
            nc.scalar.activation(attn[:], lgt[:], AF.Exp, accum_out=rsum[:])
            nc.vector.reciprocal(rsum[:], rsum[:])
            nc.vector.tensor_scalar_mul(attn[:], attn[:], rsum[:])
            keep_warm()

            # E^T = (Wp blockdiag(attn) Wv)^T, as bf16 for the bulk apply
            at_ps = psB2.tile([P, P], F32, tag="psB2")
            nc.tensor.transpose(at_ps[:], attn[:], id_f32[:])
            atT = small.tile([P, P], BF, tag="atT")
            nc.vector.tensor_copy(out=atT[:], in_=at_ps[:])
            keep_warm()
            B_ps = psB2.tile([P, P], F32, tag="psB2")
            nc.tensor.matmul(B_ps[:], atT[:], wv_sb[:], start=True, stop=True)
            B_sb = small.tile([P, P], BF, tag="B_sb")
            nc.vector.tensor_copy(out=B_sb[:], in_=B_ps[:])
            keep_warm()
            ET_ps = psB2.tile([P, P], F32, tag="psB2")
            nc.tensor.matmul(ET_ps[:], B_sb[:], wp_sb[:], start=True, stop=True)
            ET_sb = consts.tile([P, P], BF, tag="ET")
            nc.vector.tensor_copy(out=ET_sb[:], in_=ET_ps[:])

            # ---- apply E to xl, stream out (copies alternate DVE/ACT) ----
            for g in range(8):
                y_sb = y_pool.tile([P, 4, CHUNK], BF, tag="ysb")
                for j in range(4):
                    cidx = 4 * g + j
                    ps_y = ps512.tile([P, CHUNK], F32, tag="ps512")
                    nc.tensor.matmul(ps_y[:], ET_sb[:], xl_tiles[cidx][:],
                                     start=True, stop=True)
                    if j % 2 == 0:
                        nc.vector.tensor_copy(out=y_sb[:, j, :], in_=ps_y[:])
                    else:
                        nc.scalar.copy(out=y_sb[:, j, :], in_=ps_y[:])
                nc.sync.dma_start(out=out[:, g * 4 * CHUNK:(g + 1) * 4 * CHUNK], in_=y_sb[:])

    nc.compile()
    return nc


_CACHE = {}


def _get_nc():
    if "nc" not in _CACHE:
        _CACHE["nc"] = _build()
    return _CACHE["nc"]


def prep_inputs(x, w_local, w_qkv, w_proj):
    bf = ml_dtypes.bfloat16
    B = x.shape[0]
    xp = np.zeros((B, P, HP, WP), dtype=bf)
    xp[:, :, 1:H + 1, 1:W + 1] = x.astype(bf)
    # wl[i, t, o] = w_local[o, i, ky, kx]
    wl = np.ascontiguousarray(np.transpose(w_local, (1, 2, 3, 0)).reshape(P, 9, P)).astype(bf)
    wqk = np.ascontiguousarray(w_qkv[:2 * P].T).astype(bf)    # [i, o] o: q|k
    wv = np.ascontiguousarray(w_qkv[2 * P:3 * P]).astype(bf)  # [d, i]
    wp = np.ascontiguousarray(w_proj.T).astype(bf)            # [c, o]
    return [
        {"xp": xp[b], "wl": wl, "wqk": wqk, "wv": wv, "wp": wp}
        for b in range(B)
    ]


def kernel(x, w_local, w_qkv, w_proj):
    x = np.asarray(x, dtype=np.float32)
    w_local = np.asarray(w_local, dtype=np.float32)
    w_qkv = np.asarray(w_qkv, dtype=np.float32)
    w_proj = np.asarray(w_proj, dtype=np.float32)
    B = x.shape[0]

    in_maps = prep_inputs(x, w_local, w_qkv, w_proj)
    from concourse.bass_utils import run_bass_kernel_spmd
    res = run_bass_kernel_spmd(_get_nc(), in_maps, core_ids=list(range(B)))
    y = np.stack([res.results[b]["out"].astype(np.float32).reshape(P, H, W)
                  for b in range(B)])
    return y



# revision 2
# speedup vs baseline: 1.0073x; 1.0073x over previous
"""Trainium2 Bass kernel: conv3x3 + channel attention (4 heads) + 1x1 proj.

Data-parallel over batch: 8 batch elements -> 8 NeuronCores, no collectives.

Algorithm per core (one batch element, C=128, H=W=128, N=H*W=16384):
  xl = conv3x3(x)                                   # 9 shifted matmuls, bf16
  G  = xl @ xl^T          [128,128]                 # xlT via PE transpose
  # channel attention factors through G:
  #   A12 = G @ [Wq^T | Wk^T]                 [i', 256]
  #   nq2[c] = sum_i A12[i,c]*WqT[i,c]  (elementwise + ones-matmul -> row)
  #   qk = A1^T-contract: (Wq G Wk^T)[c,d]
  #   logits = qk * rq[c] * rk[d] + mask ; e = exp(logits), rs = 1/rowsum
  #   E^T = Wv^T attn^T Wp^T  (attn = e * rs folded into Wp row-scale)
  #   y = E @ xl                                    # one combined 128x128 matrix
Bulk matmuls bf16; softmax fp32. ACT uses only {Copy, Ln, Exp} so a single
activation-table set (natural_log_exp_and_others) is loaded once.
Filler matmuls (213ns each) are threaded through the serial attention chain at
PE-queue positions that keep the PE busy so the HAM clock gate stays at 8/8.
"""

import numpy as np
import ml_dtypes

import concourse.bass as bass
import concourse.mybir as mybir
import concourse.tile as tile
from concourse import bacc
from concourse.masks import make_identity

# Make Exp/Ln resolve to the combined "natural_log_exp_and_others" activation
# table set so the whole kernel needs exactly one ACT_TABLE_LOAD (Copy is in
# every set). The default first-match choice picks disjoint sets for Exp and
# Ln, costing two ~1.3us mid-kernel table reloads on the critical path.
_ORIG_GAT = bacc.get_activation_tables


def _gat_one_set(arch):
    tables = _ORIG_GAT(arch)
    for name, fns in tables.items():
        if name != "natural_log_exp_and_others":
            fns.discard(mybir.ActivationFunctionType.Exp)
            fns.discard(mybir.ActivationFunctionType.Ln)
    return tables


bacc.get_activation_tables = _gat_one_set

P = 128
H = W = 128
HP, WP = H + 2, W + 2          # zero-padded input
NPOS = H * W                   # 16384
CHUNK = 512                    # spatial chunk (4 rows)
HCH = CHUNK // 2
NCHUNK = NPOS // CHUNK         # 32
NSTRIP = 4                     # input strips (with halo)
STRIP_ROWS = 34                # 32 rows + 2 halo
NH = 4                         # heads
CH = 32                        # channels per head
BF = mybir.dt.bfloat16
F32 = mybir.dt.float32
AX = mybir.AxisListType
AF = mybir.ActivationFunctionType
OP = mybir.AluOpType
MASK_NEG = -1.0e12
G_LAG = 2                      # chunks of lag before G matmuls consume xlT


def _build():
    nc = bacc.Bacc()
    xp = nc.declare_dram_parameter("xp", [P, HP, WP], BF, isOutput=False)
    wl = nc.declare_dram_parameter("wl", [P, 9, P], BF, isOutput=False)
    wqk = nc.declare_dram_parameter("wqk", [P, 2 * P], BF, isOutput=False)
    wv = nc.declare_dram_parameter("wv", [P, P], BF, isOutput=False)
    wp = nc.declare_dram_parameter("wp", [P, P], BF, isOutput=False)
    out = nc.declare_dram_parameter("out", [P, NPOS], BF, isOutput=True)

    with tile.TileContext(nc) as tc:
        with (
            tc.tile_pool(name="consts", bufs=1) as consts,
            tc.tile_pool(name="xstrip", bufs=NSTRIP) as xstrip_pool,
            tc.tile_pool(name="xl", bufs=NCHUNK) as xl_pool,
            tc.tile_pool(name="xlt", bufs=G_LAG + 2) as xlt_pool,
            tc.tile_pool(name="small", bufs=1) as small,
            tc.tile_pool(name="ysb", bufs=4) as y_pool,
            tc.tile_pool(name="ps512", bufs=3, space="PSUM") as ps512,
            tc.tile_pool(name="psT", bufs=2, space="PSUM") as psT,
            tc.tile_pool(name="psG", bufs=1, space="PSUM") as psG,
            tc.tile_pool(name="psA", bufs=2, space="PSUM") as psA,
        ):
            # ---- conv weights + input strips first so their DMAs lead the queue ----
            # First strip is split in half (18 rows each) so the first conv
            # chunks can start ~2us earlier; chunks 0-3 need xp rows 0-17 only.
            strip_defs = [(0, 18), (16, 18)] + [
                (32 * s, STRIP_ROWS) for s in range(1, NSTRIP)
            ]
            st0 = xstrip_pool.tile([P, 18, WP], BF, tag="xstrip0")
            nc.sync.dma_start(out=st0[:], in_=xp[:, 0:18, :])
            wl_sb = consts.tile([P, 9, P], BF, tag="wl")
            nc.sync.dma_start(out=wl_sb[:], in_=wl[:])
            xstrips = [st0]
            for r0, nr in strip_defs[1:]:
                st = xstrip_pool.tile([P, STRIP_ROWS, WP], BF, tag="xstrip")
                nc.sync.dma_start(out=st[:, 0:nr, :], in_=xp[:, r0: r0 + nr, :])
                xstrips.append(st)

            def conv_src(c):
                # returns (strip tile, local row base) for output chunk c
                if c < 4:
                    return xstrips[0], 4 * c
                if c < 8:
                    return xstrips[1], 4 * c - 16
                s = c // 8
                return xstrips[s + 1], 4 * (c % 8)
            wqk_sb = consts.tile([P, 2 * P], BF, tag="wqk")
            nc.sync.dma_start(out=wqk_sb[:], in_=wqk[:])
            wv_sb = consts.tile([P, P], BF, tag="wv")
            nc.sync.dma_start(out=wv_sb[:], in_=wv[:])
            wp_sb = consts.tile([P, P], BF, tag="wp")
            nc.sync.dma_start(out=wp_sb[:], in_=wp[:])

            junk = consts.tile([P, CHUNK], BF, tag="junk")
            nc.vector.memset(junk[:], 0.125)

            def keep_warm(n=1):
                # Dependency-free 512-col matmuls into free psT slots.  They
                # sit in the PE FIFO where emitted and drain at 213ns each,
                # keeping the HAM clock gate open through serial DVE/ACT work.
                for _ in range(n):
                    pw = psT.tile([P, CHUNK], F32, tag="psT")
                    nc.tensor.matmul(pw[:], junk[:, 0:P], junk[:],
                                     start=True, stop=True)

            # ---- single ACT table load (set: natural_log_exp_and_others) ----
            tl = small.tile([P, 1], F32, tag="tl")
            nc.vector.memset(tl[:], 1.0)
            nc.scalar.activation(tl[:], tl[:], AF.Exp)

            id_bf = consts.tile([P, P], BF, tag="id_bf")
            make_identity(nc, id_bf[:])
            mask_sb = consts.tile([P, P], F32, tag="mask")
            nc.vector.memset(mask_sb[:], MASK_NEG)
            for h in range(NH):
                nc.vector.memset(mask_sb[h * CH:(h + 1) * CH, h * CH:(h + 1) * CH], 0.0)
            ones_col = consts.tile([P, 1], BF, tag="ones_col")
            nc.vector.memset(ones_col[:], 1.0)
            ones_row = consts.tile([1, P], BF, tag="ones_row")
            nc.vector.memset(ones_row[:], 1.0)
            one1 = consts.tile([1, 1], BF, tag="one1")
            nc.vector.memset(one1[:], 1.0)

            # ---- main loop: conv; xlT via PE transpose; lagged G accumulation ----
            # PE order per chunk: conv(c) x9, G(c-2) x4 (deps long ready, fill
            # the xl-cast latency), T(c) x4.  xl cast is split across DVE+ACT
            # so the T matmuls' stationary operand is ready ~250ns sooner.
            G_ps = psG.tile([P, P], F32, tag="G")
            xl_tiles = []
            xlt_tiles = []

            def g_mms(ci):
                xlt4 = xlt_tiles[ci]
                for sub in range(4):
                    idx = ci * 4 + sub
                    nc.tensor.matmul(G_ps[:], xlt4[:, sub, :], xlt4[:, sub, :],
                                     start=(idx == 0), stop=(idx == 4 * NCHUNK - 1))

            for c in range(NCHUNK):
                strip, lb = conv_src(c)
                ps_conv = ps512.tile([P, CHUNK], F32, tag="ps512")
                for t in range(9):
                    ky, kx = divmod(t, 3)
                    rhs = strip[:, lb + ky: lb + ky + 4, kx: kx + W]
                    nc.tensor.matmul(ps_conv[:], wl_sb[:, t, :], rhs,
                                     start=(t == 0), stop=(t == 8))
                xl_c = xl_pool.tile([P, CHUNK], BF, tag="xl")
                nc.vector.tensor_copy(out=xl_c[:, 0:HCH], in_=ps_conv[:, 0:HCH])
                nc.scalar.copy(out=xl_c[:, HCH:CHUNK], in_=ps_conv[:, HCH:CHUNK])
                xl_tiles.append(xl_c)
                if c >= G_LAG:
                    g_mms(c - G_LAG)
                ps_t4 = psT.tile([P, 4, P], F32, tag="psT")
                for sub in range(4):
                    nc.tensor.matmul(ps_t4[:, sub, :], xl_c[:, sub * P:(sub + 1) * P],
                                     id_bf[:], start=True, stop=True)
                xlt4 = xlt_pool.tile([P, 4, P], BF, tag="xlt")
                if c % 2 == 0:
                    nc.vector.tensor_copy(out=xlt4[:], in_=ps_t4[:])
                else:
                    nc.scalar.copy(out=xlt4[:], in_=ps_t4[:])
                xlt_tiles.append(xlt4)
            for ci in range(NCHUNK - G_LAG, NCHUNK):
                g_mms(ci)

            # ---- attention: one short serial chain, PE kept warm by fillers ----
            G_sb = small.tile([P, P], BF, tag="G_sb")
            nc.vector.tensor_copy(out=G_sb[:], in_=G_ps[:])
            # A12[i', o] = (G [Wq^T | Wk^T])[i', o], o = q|k out channel
            A12_ps = psA.tile([P, 2 * P], F32, tag="psA")
            nc.tensor.matmul(A12_ps[:], G_sb[:], wqk_sb[:], start=True, stop=True)
            keep_warm(2)
            # nprod[i', o] = A12[i', o] * Wqk^T[i', o]; summing over i' gives
            # diag(Wq G Wq^T) | diag(Wk G Wk^T) = squared q|k norms.
            nprod_sb = small.tile([P, 2 * P], BF, tag="nprod")
            nc.vector.tensor_tensor(nprod_sb[:], A12_ps[:], wqk_sb[:], OP.mult)
            A1_sb = small.tile([P, P], BF, tag="A1_sb")
            nc.scalar.copy(out=A1_sb[:], in_=A12_ps[:, 0:P])
            n2_ps = psA.tile([1, 2 * P], F32, tag="psA")
            nc.tensor.matmul(n2_ps[:], ones_col[:], nprod_sb[:], start=True, stop=True)
            keep_warm(1)
            # qk[c, d] = (Wq G Wk^T)[c, d]
            qk_ps = psA.tile([P, P], F32, tag="psA")
            nc.tensor.matmul(qk_ps[:], A1_sb[:], wqk_sb[:, P:2 * P],
                             start=True, stop=True)
            keep_warm(2)
            # r = 1/sqrt(n2) as a row [1, 256]: rq | rk  (bf16 is plenty here)
            ln_row = small.tile([1, 2 * P], F32, tag="ln_row")
            nc.scalar.activation(ln_row[:], n2_ps[:], AF.Ln)
            r_row = small.tile([1, 2 * P], BF, tag="r_row")
            nc.scalar.activation(r_row[:], ln_row[:], AF.Exp, scale=-0.5)
            # additive mask applied pre-scale: -1e12 * rq * rk is still << 0
            qk_m = small.tile([P, P], F32, tag="qk_m")
            nc.vector.tensor_tensor(qk_m[:], qk_ps[:], mask_sb[:], OP.add)
            # rq as a per-partition column; rk broadcast down partitions
            rq_ps = psA.tile([P, 1], F32, tag="psA")
            nc.tensor.matmul(rq_ps[:], r_row[0:1, 0:P], one1[:], start=True, stop=True)
            RK_ps = psA.tile([P, P], F32, tag="psA")
            nc.tensor.matmul(RK_ps[:], ones_row[:], r_row[0:1, P:2 * P],
                             start=True, stop=True)
            keep_warm(6)
            rq_sb = small.tile([P, 1], F32, tag="rq_sb")
            nc.vector.tensor_copy(out=rq_sb[:], in_=rq_ps[:])
            # logits = (RK * rq) * qk_m ; exp with fp32 row-sum accumulator
            L_sb = small.tile([P, P], F32, tag="L_sb")
            nc.vector.scalar_tensor_tensor(
                out=L_sb[:], in0=RK_ps[:], scalar=rq_sb[:], in1=qk_m[:],
                op0=OP.mult, op1=OP.mult)
            e_sb = small.tile([P, P], BF, tag="e_sb")
            rsum = small.tile([P, 1], F32, tag="rsum")
            nc.scalar.activation(e_sb[:], L_sb[:], AF.Exp, accum_out=rsum[:])
            nc.vector.reciprocal(rsum[:], rsum[:])
            # fold softmax normalization into Wp^T row scale:
            # M1[d, o] = sum_c e[c, d] * rs[c] * Wp^T[c, o] = (attn^T Wp^T)[d, o]
            wp_s = small.tile([P, P], BF, tag="wp_s")
            nc.vector.tensor_scalar_mul(wp_s[:], wp_sb[:], rsum[:])
            M1_ps = psA.tile([P, P], F32, tag="psA")
            nc.tensor.matmul(M1_ps[:], e_sb[:], wp_s[:], start=True, stop=True)
            keep_warm(1)
            M1_sb = small.tile([P, P], BF, tag="M1_sb")
            nc.vector.tensor_copy(out=M1_sb[:], in_=M1_ps[:])
            # E^T[i, o] = sum_d Wv[d, i] M1[d, o]
            ET_ps = psA.tile([P, P], F32, tag="psA")
            nc.tensor.matmul(ET_ps[:], wv_sb[:], M1_sb[:], start=True, stop=True)
            keep_warm(1)
            ET_sb = consts.tile([P, P], BF, tag="ET")
            nc.vector.tensor_copy(out=ET_sb[:], in_=ET_ps[:])

            # ---- apply E to xl, stream out (copies alternate DVE/ACT) ----
            # 5 PSUM banks rotate (3 from ps512 + 2 from psT); DMA per 2 chunks
            # so the output stream starts early and stays DMA-bound.
            for g in range(NCHUNK // 2):
                y_sb = y_pool.tile([P, 2, CHUNK], BF, tag="ysb")
                for j in range(2):
                    cidx = 2 * g + j
                    k = cidx % 5
                    if k < 3:
                        ps_y = ps512.tile([P, CHUNK], F32, tag="ps512")
                    else:
                        ps_y = psT.tile([P, CHUNK], F32, tag="psT")
                    nc.tensor.matmul(ps_y[:], ET_sb[:], xl_tiles[cidx][:],
                                     start=True, stop=True)
                    if cidx % 2 == 0:
                        nc.vector.tensor_copy(out=y_sb[:, j, :], in_=ps_y[:])
                    else:
                        nc.scalar.copy(out=y_sb[:, j, :], in_=ps_y[:])
                nc.sync.dma_start(out=out[:, g * 2 * CHUNK:(g + 1) * 2 * CHUNK], in_=y_sb[:])

    nc.compile()
    return nc


_CACHE = {}


def _get_nc():
    if "nc" not in _CACHE:
        _CACHE["nc"] = _build()
    return _CACHE["nc"]


def prep_inputs(x, w_local, w_qkv, w_proj):
    bf = ml_dtypes.bfloat16
    B = x.shape[0]
    xp = np.zeros((B, P, HP, WP), dtype=bf)
    xp[:, :, 1:H + 1, 1:W + 1] = x.astype(bf)
    # wl[i, t, o] = w_local[o, i, ky, kx]
    wl = np.ascontiguousarray(np.transpose(w_local, (1, 2, 3, 0)).reshape(P, 9, P)).astype(bf)
    wqk = np.ascontiguousarray(w_qkv[:2 * P].T).astype(bf)    # [i, o] o: q|k
    wv = np.ascontiguousarray(w_qkv[2 * P:3 * P]).astype(bf)  # [d, i]
    wp = np.ascontiguousarray(w_proj.T).astype(bf)            # [c, o]
    return [
        {"xp": xp[b], "wl": wl, "wqk": wqk, "wv": wv, "wp": wp}
        for b in range(B)
    ]


def kernel(x, w_local, w_qkv, w_proj):
    x = np.asarray(x, dtype=np.float32)
    w_local = np.asarray(w_local, dtype=np.float32)
    w_qkv = np.asarray(w_qkv, dtype=np.float32)
    w_proj = np.asarray(w_proj, dtype=np.float32)
    B = x.shape[0]

    in_maps = prep_inputs(x, w_local, w_qkv, w_proj)
    from concourse.bass_utils import run_bass_kernel_spmd
    res = run_bass_kernel_spmd(_get_nc(), in_maps, core_ids=list(range(B)))
    y = np.stack([res.results[b]["out"].astype(np.float32).reshape(P, H, W)
                  for b in range(B)])
    return y
